# revision 7
# baseline (speedup 1.0000x reference)
"""HadLinear TRN2 kernel: out = fwht_1024blocks(x)/sqrt(1024) @ W.T

Math: fwht on 1024-blocks is x @ H_bd, H_bd = blockdiag(H_1024 x4),
H_1024 = H_8 (x) H_128 (natural order, k = j*128 + p). The 1/sqrt(1024)
= 2^-5 scale is folded into H_128 (exact in bf16).

Sharding: data-parallel row shard of x (2048 rows/core). Host passes all
tensors in device-friendly permuted layouts (pure layout changes) so
every DMA is a fully contiguous >=0.5MB block transfer. x and W are
loaded with casting DMAs (SWDGE, f32 DRAM -> bf16 SBUF) so no engine
spends time on dtype conversion.

Per core (M_CORE=2048 rows, 2 halves of 1024):
  Phase A (unit = 256-row m-strip x one 1024-k-block): PE computes 8
  PSUM slices with the first one (steady) or two (opening) H8 butterfly
  stages folded into +/-H128 PSUM accumulation; ACT evicts to bf16; the
  remaining H8 stages are large strided add/sub ops on GpSimd/DVE using
  a bit-rotation layout (every stage is the same op shape).
  Phase B: per 128-row output group g, C[g] = sum_kt A[kt].T @ W[kt,n]
  accumulated k-contiguously in ONE PSUM bank (32 matmuls, N=512), ACT
  evicts to SBUF, 1MB y DMA per 4 groups.

Schedule (emission order == per-engine issue order): the opening strip
interleaves phase-A units with chunk-wise (kt-outer) phase-B matmuls;
later strips are k-contiguous per group with the next strip's W chunks
and the next half's A units woven between groups, so at steady state
the PE runs back-to-back matmuls with zero gaps.

Self-contained: hardcodes B=4, S=4096, D_in=D_out=4096, 8 cores.
"""

import numpy as np
import ml_dtypes

import concourse.bacc as bacc
import concourse.mybir as mybir
import concourse.tile as tile
from concourse.bass_utils import run_bass_kernel_spmd

P = 128
N_CORES = 8
B_FULL, S_FULL, D = 4, 4096, 4096
M_FULL = B_FULL * S_FULL          # 16384 rows total
M_CORE = M_FULL // N_CORES        # 2048 rows per core
NBLK = 4                          # 1024-wide hadamard blocks per row
ASTRIP = 256                      # phase A m-strip width
NSTRIP = 512                      # phase B out-feature strip width
NS_PER_HALF = 8                   # strips per half (4096/512)
MS_PER_HALF = 4                   # 256-row m-strips per half
WCH = 4                           # k-tiles per W chunk -> 8 chunks/strip


def _h128_np():
    """H_128 (natural order) scaled by 1/sqrt(1024) = 2^-5; exact in bf16."""
    h = np.array([[(-1.0) ** bin(i & j).count("1") for j in range(P)]
                  for i in range(P)])
    return (h / 32.0).astype(ml_dtypes.bfloat16)


def build_nc():
    f32, bf16 = mybir.dt.float32, mybir.dt.bfloat16
    nc = bacc.Bacc(None, target_bir_lowering=False, debug=False)

    # x: 32 units of [P, 8j, 256m]; unit u = (half*4+msl)*4 + blk
    xd = nc.declare_dram_parameter("xd", [32, P, 2048], f32, isOutput=False)
    # W: 64 chunks of [P, 4c, 512n]; chunk = ns*8 + ch (kt = ch*4 + c)
    wd = nc.declare_dram_parameter("wd", [64, P, 2048], f32, isOutput=False)
    h = nc.declare_dram_parameter("h", [2 * P, P], bf16, isOutput=False)
    # y: 32 blocks of [P, 4i, 512n]; block = half*16 + ns*2 + gset
    yd = nc.declare_dram_parameter("yd", [32, P, 2048], f32, isOutput=True)

    with tile.TileContext(nc) as tc:
        with (
            tc.tile_pool(name="const", bufs=1) as constp,
            tc.tile_pool(name="xb", bufs=3) as xbp,
            tc.tile_pool(name="vv", bufs=2) as vp,
            tc.tile_pool(name="ss", bufs=3) as sp,
            tc.tile_pool(name="apool", bufs=23) as apool,
            tc.tile_pool(name="wbf", bufs=16) as wbfp,
            tc.tile_pool(name="outp", bufs=2) as outp,
            tc.tile_pool(name="psV", bufs=2, space="PSUM") as psV,
            tc.tile_pool(name="psC", bufs=4, space="PSUM") as psC,
        ):
            h128p = constp.tile([P, P], bf16, tag="hp", name="h128p")
            nc.sync.dma_start(out=h128p[:], in_=h[0:P, :])
            h128n = constp.tile([P, P], bf16, tag="hn", name="h128n")
            nc.sync.dma_start(out=h128n[:], in_=h[P:2 * P, :])

            A = {}      # (half, msl, blk) -> [P, 8b, 256m] bf16
            XB = {}     # loaded-but-not-computed units
            WB = {}     # (half, ns, ch) -> [P, 4c, 512n] bf16

            def aload(half, msl, blk):
                u = (half * MS_PER_HALF + msl) * NBLK + blk
                xb = xbp.tile([P, 2048], bf16, tag="xb", name=f"xb_{u}")
                nc.gpsimd.dma_start(out=xb[:], in_=xd[u])   # casting DMA
                XB[(half, msl, blk)] = xb

            def acompute(half, msl, blk, fold2=False):
                u = (half * MS_PER_HALF + msl) * NBLK + blk
                xb = XB.pop((half, msl, blk))
                xbj = lambda j: xb[:, j * ASTRIP:(j + 1) * ASTRIP]
                pva = psV.tile([P, 4, ASTRIP], f32, tag="V", name=f"pva_{u}")
                pvb = psV.tile([P, 4, ASTRIP], f32, tag="V", name=f"pvb_{u}")
                if fold2:
                    # first TWO H8 stages folded into PE accumulation:
                    # V[d3*4+b1*2+b2] = sum_{d1,d2} +/- H@x[d1*4+d2*2+d3]
                    for jp in range(8):
                        d3, r = divmod(jp, 4)
                        b1, b2 = divmod(r, 2)
                        dst = (pva if jp < 4 else pvb)[:, jp % 4, :]
                        terms = [((b1 & d1) ^ (b2 & d2), d1 * 4 + d2 * 2 + d3)
                                 for d1 in range(2) for d2 in range(2)]
                        for t, (sgn, j) in enumerate(terms):
                            nc.tensor.matmul(
                                dst, lhsT=(h128p if sgn == 0 else h128n)[:],
                                rhs=xbj(j), start=(t == 0), stop=(t == 3))
                else:
                    # first H8 stage folded: V[2q+b1] = H@x[q] +/- H@x[q+4]
                    for jp in range(8):
                        q, b1 = divmod(jp, 2)
                        dst = (pva if jp < 4 else pvb)[:, jp % 4, :]
                        nc.tensor.matmul(dst, lhsT=h128p[:], rhs=xbj(q),
                                         start=True, stop=False)
                        nc.tensor.matmul(dst,
                                         lhsT=(h128p if b1 == 0 else h128n)[:],
                                         rhs=xbj(q + 4), start=False, stop=True)
                v = vp.tile([P, 8, ASTRIP], bf16, tag="v", name=f"v_{u}")
                nc.scalar.copy(out=v[:, 0:4, :], in_=pva[:])
                nc.scalar.copy(out=v[:, 4:8, :], in_=pvb[:])

                # rotation butterfly: out[2q+0]=in[q]+in[q+4],
                # out[2q+1]=in[q]-in[q+4]: each pass shifts the index bits
                # left, ending at (b1,b2,b3) = natural H8 index
                def stage(dst, src, add_op, sub_op):
                    in0, in1 = src[:, 0:4, :], src[:, 4:8, :]
                    o = dst.rearrange("p (q b) m -> p q b m", b=2)
                    add_op(out=o[:, :, 0, :], in0=in0, in1=in1)
                    sub_op(out=o[:, :, 1, :], in0=in0, in1=in1)

                a = apool.tile([P, 8, ASTRIP], bf16, tag="A", name=f"a_{u}")
                if fold2:
                    # single remaining stage; DVE add + GpSimd sub in parallel
                    stage(a, v, nc.vector.tensor_add, nc.gpsimd.tensor_sub)
                else:
                    s2 = sp.tile([P, 8, ASTRIP], bf16, tag="s", name=f"s2_{u}")
                    stage(s2, v, nc.gpsimd.tensor_add, nc.gpsimd.tensor_sub)
                    stage(a, s2, nc.vector.tensor_add, nc.vector.tensor_sub)
                A[(half, msl, blk)] = a

            def loadw(s, ch):
                half, ns = divmod(s, NS_PER_HALF)
                wb = wbfp.tile([P, WCH, NSTRIP], bf16, tag="wbf",
                               name=f"wb_{s}_{ch}")
                nc.gpsimd.dma_start(                        # casting DMA
                    out=wb[:],
                    in_=wd[ns * 8 + ch].rearrange("p (c n) -> p c n", c=WCH))
                WB[(half, ns, ch)] = wb

            def lhsT_of(half, g, kt):
                blk, b = divmod(kt, 8)
                msl, sub = divmod(g, 2)
                return A[(half, msl, blk)][:, b, sub * P:(sub + 1) * P]

            def bmm(pc, half, ns, g, kt):
                nc.tensor.matmul(
                    pc[:], lhsT=lhsT_of(half, g, kt),
                    rhs=WB[(half, ns, kt // WCH)][:, kt % WCH, :],
                    start=(kt == 0), stop=(kt == 31))

            couts = {}

            def evict_group(half, ns, g, pc):
                gset, i = divmod(g, 4)
                if i == 0:
                    couts[gset % 2] = outp.tile(
                        [P, 4, NSTRIP], f32, tag="out",
                        name=f"co_{half}_{ns}_{gset}")
                co = couts[gset % 2]
                nc.scalar.copy(out=co[:, i, :], in_=pc[:])
                if i == 3:
                    nc.sync.dma_start(
                        out=yd[half * 16 + ns * 2 + gset].rearrange(
                            "p (i n) -> p i n", i=4),
                        in_=co[:])

            # ---------------- opening strip (half 0, ns 0) ----------------
            aload(0, 0, 0)
            loadw(0, 0)
            aload(0, 1, 0)
            loadw(0, 1)
            for gset in range(2):
                msl0, msl1 = (0, 1) if gset == 0 else (2, 3)
                pcs = {}
                for slot in range(5):
                    if slot < 4:
                        acompute(0, msl0, slot, fold2=True)
                        acompute(0, msl1, slot, fold2=True)
                        if slot < 3:
                            aload(0, msl0, slot + 1)
                            aload(0, msl1, slot + 1)
                        elif gset == 0:
                            aload(0, 2, 0)
                            aload(0, 3, 0)
                        if gset == 0 and slot < 3:
                            loadw(0, 2 * slot + 2)
                            loadw(0, 2 * slot + 3)
                        elif gset == 1:
                            loadw(1, 2 * slot)      # strip 1 prefetch
                            loadw(1, 2 * slot + 1)
                            if slot == 3:
                                loadw(1, 6)
                                loadw(1, 7)
                    if slot > 0:
                        for g in (0, 1, 2, 3) if gset == 0 else (4, 5, 6, 7):
                            if slot == 1:
                                pcs[g] = psC.tile([P, NSTRIP], f32, tag="C",
                                                  name=f"pc_0_0_{g}")
                            for kt in range((slot - 1) * 8, slot * 8):
                                bmm(pcs[g], 0, 0, g, kt)
                for g in (0, 1, 2, 3) if gset == 0 else (4, 5, 6, 7):
                    evict_group(0, 0, g, pcs[g])

            # ---------------- steady strips ----------------
            # A(1) weave: units #1..16 in B-consumption order
            a1_units = []
            for mpair in ((0, 1), (2, 3)):
                for blk in range(NBLK):
                    a1_units.append((mpair[0], blk))
                    a1_units.append((mpair[1], blk))
            weave = {}   # (s, g) -> list of thunks

            def add_weave(s, g, fn):
                weave.setdefault((s, g), []).append(fn)

            # loads at s6 g0..g7 / s7 g0..g7; computes one group later
            for k in range(16):
                s = 6 + k // 8
                g = k % 8
                msl, blk = a1_units[k]
                add_weave(s, g, (lambda m, b: lambda: aload(1, m, b))(msl, blk))
                cs, cg = (s, g + 1) if g < 7 else (s + 1, 0)
                add_weave(cs, cg,
                          (lambda m, b: lambda: acompute(1, m, b))(msl, blk))

            for s in range(1, 16):
                half, ns = divmod(s, NS_PER_HALF)
                for g in range(8):
                    if s < 15:
                        loadw(s + 1, g)
                    for fn in weave.get((s, g), ()):
                        fn()
                    pc = psC.tile([P, NSTRIP], f32, tag="C",
                                  name=f"pc_{half}_{ns}_{g}")
                    for kt in range(32):
                        bmm(pc, half, ns, g, kt)
                    evict_group(half, ns, g, pc)

    nc.compile()
    return nc


_CACHE = {}


def _get_nc():
    if "nc" not in _CACHE:
        _CACHE["nc"] = build_nc()
    return _CACHE["nc"]


def _prep_x(xc):
    """[2048, 4096] f32 -> [32, 128, 2048]: unit (ms, blk), [p, j, m]."""
    return np.ascontiguousarray(
        xc.reshape(8, 256, 4, 8, 128).transpose(0, 2, 4, 3, 1)
    ).reshape(32, 128, 2048)


def _prep_w(w):
    """[4096, 4096] (n,k) f32 -> [64, 128, 2048]: chunk (ns, ch), [p, c, n]."""
    return np.ascontiguousarray(
        w.reshape(8, 512, 8, 4, 128).transpose(0, 2, 4, 3, 1)
    ).reshape(64, 128, 2048)


def _unprep_y(ydv):
    """[32, 128, 2048] f32 -> [2048, 4096]."""
    return np.ascontiguousarray(
        ydv.reshape(2, 8, 2, 128, 4, 512).transpose(0, 2, 4, 3, 1, 5)
    ).reshape(2048, 4096)


def run(x, weight, trace=False):
    assert x.shape == (B_FULL, S_FULL, D) and weight.shape == (D, D)
    nc = _get_nc()
    xf = np.asarray(x, dtype=np.float32).reshape(M_FULL, D)
    wv = np.asarray(weight, dtype=np.float32)
    wdv = _prep_w(wv)
    h1 = np.asarray(_h128_np())
    hh = np.concatenate([h1, -h1], axis=0)
    in_maps = [
        {"xd": _prep_x(xf[c * M_CORE:(c + 1) * M_CORE]),
         "wd": wdv, "h": hh}
        for c in range(N_CORES)
    ]
    res = run_bass_kernel_spmd(nc, in_maps, core_ids=list(range(N_CORES)),
                               trace=trace)
    yv = np.concatenate([_unprep_y(r["yd"]) for r in res.results], axis=0)
    return yv.reshape(B_FULL, S_FULL, D), res


def kernel(x, weight):
    return run(x, weight)[0]


# revision 14
# speedup vs baseline: 1.1642x; 1.1642x over previous
"""HadLinear TRN2 kernel: out = fwht_1024blocks(x)/sqrt(1024) @ W.T

Math: fwht on 1024-blocks is x @ H_bd, H_bd = blockdiag(H_1024 x4),
H_1024 = H_8 (x) H_128 (natural order, k = j*128 + p). The 1/sqrt(1024)
= 2^-5 scale is folded into H_128 (exact in bf16).

Sharding: data-parallel row shard of x (2048 rows/core). Host passes all
tensors in device-friendly permuted layouts (pure layout changes) so
every DMA is a fully contiguous >=0.5MB block transfer.

Per core (M_CORE=2048 rows split in 2 halves of 1024):
  Phase A (per unit = 256-row m-strip x one 1024-k-block): DVE casts x to
  bf16; PE computes V[2q+b] = H128@x[q] +/- H128@x[q+4] (first H8
  butterfly folded into PSUM accumulation via +/-H128 constants); ACT
  evicts PSUM to a bf16 [128,8,256] tile; remaining two H8 stages run as
  4 large strided add/sub ops (GpSimd stage, then DVE stage) using a
  bit-rotation layout so every stage is the same op shape.
  Phase B: per 128-row output group g, C[g] = sum_kt A[kt].T @ W[kt,n]
  accumulated k-contiguously in ONE PSUM bank (32 matmuls, N=256), ACT
  evicts to an SBUF staging tile, 0.5MB y DMA per 4 groups.

Schedule (emission order == per-engine issue order): the opening strip
interleaves phase-A units with chunk-wise (kt-outer) phase-B matmuls so
PE work starts as soon as the first x/W bytes land; every later strip is
k-contiguous per group with the next strip's W chunks and the next
half's A units woven between groups, so the PE never waits on phase A
or W loads at steady state.

Self-contained: hardcodes B=4, S=4096, D_in=D_out=4096, 8 cores.
"""

import numpy as np
import ml_dtypes

import concourse.bacc as bacc
import concourse.mybir as mybir
import concourse.tile as tile
from concourse.bass_utils import run_bass_kernel_spmd

P = 128
N_CORES = 8
B_FULL, S_FULL, D = 4, 4096, 4096
M_FULL = B_FULL * S_FULL          # 16384 rows total
M_CORE = M_FULL // N_CORES        # 2048 rows per core
HAD = 1024                        # hadamard block
NBLK = D // HAD                   # 4 k-blocks of 1024
ASTRIP = 256                      # phase A m-strip width
NSTRIP = 256                      # phase B out-feature strip width
NS_PER_HALF = 16                  # strips per half (4096/256)
MS_PER_HALF = 4                   # 256-row m-strips per half
WCH = 8                           # k-tiles per W chunk -> 4 chunks/strip


def _h128_np():
    """H_128 (natural order) scaled by 1/sqrt(1024) = 2^-5; exact in bf16."""
    h = np.array([[(-1.0) ** bin(i & j).count("1") for j in range(P)]
                  for i in range(P)])
    return (h / 32.0).astype(ml_dtypes.bfloat16)


def build_nc():
    f32, bf16 = mybir.dt.float32, mybir.dt.bfloat16
    nc = bacc.Bacc(None, target_bir_lowering=False, debug=False)

    # x: 32 units of [P, 8j, 256m]; unit u = (half*4+msl)*4 + blk
    xd = nc.declare_dram_parameter("xd", [32, P, 2048], f32, isOutput=False)
    # W: 64 chunks of [P, 8c, 256n]; chunk = ns*4 + ch (kt = ch*8 + c)
    wd = nc.declare_dram_parameter("wd", [64, P, 2048], f32, isOutput=False)
    h = nc.declare_dram_parameter("h", [2 * P, P], bf16, isOutput=False)
    # y: 64 blocks of [P, 4i, 256n]; block = half*32 + ns*2 + gset
    yd = nc.declare_dram_parameter("yd", [64, P, 1024], f32, isOutput=True)

    with tile.TileContext(nc) as tc:
        with (
            tc.tile_pool(name="const", bufs=1) as constp,
            tc.tile_pool(name="xs", bufs=3) as xsp,
            tc.tile_pool(name="xb", bufs=2) as xbp,
            tc.tile_pool(name="vv", bufs=2) as vp,
            tc.tile_pool(name="ss", bufs=3) as sp,
            tc.tile_pool(name="apool", bufs=24) as apool,
            tc.tile_pool(name="wst", bufs=2) as wstp,
            tc.tile_pool(name="wbf", bufs=8) as wbfp,
            tc.tile_pool(name="outp", bufs=2) as outp,
            tc.tile_pool(name="psV", bufs=2, space="PSUM") as psV,
            tc.tile_pool(name="psC", bufs=4, space="PSUM") as psC,
        ):
            h128p = constp.tile([P, P], bf16, tag="hp", name="h128p")
            nc.sync.dma_start(out=h128p[:], in_=h[0:P, :])
            h128n = constp.tile([P, P], bf16, tag="hn", name="h128n")
            nc.sync.dma_start(out=h128n[:], in_=h[P:2 * P, :])

            # HAM warm-up: ~3.5us of tiny matmuls during the initial DMA
            # window so the PE clock is at 8/8 when real work arrives.
            warm = psV.tile([P, 4, ASTRIP], f32, tag="V", name="warm")
            for w in range(56):
                nc.tensor.matmul(warm[:, 0, 0:64], lhsT=h128p[:],
                                 rhs=h128p[:, 0:64], start=True, stop=True)

            A = {}      # (half, msl, blk) -> [P, 8b, 256m] bf16
            XB = {}     # staging for loaded-but-not-computed units
            WB = {}     # (half, ns, ch) -> [P, 8c, 256n] bf16

            def aload(half, msl, blk, split=False):
                u = (half * MS_PER_HALF + msl) * NBLK + blk
                xs = xsp.tile([P, 2048], f32, tag="xs", name=f"xs_{u}")
                xb = xbp.tile([P, 2048], bf16, tag="xb", name=f"xb_{u}")
                if split:
                    # halve time-to-first-matmul at kernel start
                    nc.sync.dma_start(out=xs[:, 0:1024], in_=xd[u][:, 0:1024])
                    nc.vector.tensor_copy(out=xb[:, 0:1024],
                                          in_=xs[:, 0:1024])
                    nc.sync.dma_start(out=xs[:, 1024:2048],
                                      in_=xd[u][:, 1024:2048])
                    nc.vector.tensor_copy(out=xb[:, 1024:2048],
                                          in_=xs[:, 1024:2048])
                else:
                    nc.sync.dma_start(out=xs[:], in_=xd[u])
                    nc.vector.tensor_copy(out=xb[:], in_=xs[:])
                XB[(half, msl, blk)] = xb

            def acompute(half, msl, blk, fold2=False):
                u = (half * MS_PER_HALF + msl) * NBLK + blk
                xb = XB.pop((half, msl, blk))
                xbj = lambda j: xb[:, j * ASTRIP:(j + 1) * ASTRIP]
                pva = psV.tile([P, 4, ASTRIP], f32, tag="V", name=f"pva_{u}")
                pvb = psV.tile([P, 4, ASTRIP], f32, tag="V", name=f"pvb_{u}")
                if fold2:
                    # first TWO H8 stages folded into PE accumulation:
                    # V[d3*4+b1*2+b2] = sum_{d1,d2} +/- H@x[d1*4+d2*2+d3]
                    for jp in range(8):
                        d3, r = divmod(jp, 4)
                        b1, b2 = divmod(r, 2)
                        dst = (pva if jp < 4 else pvb)[:, jp % 4, :]
                        terms = [((b1 & d1) ^ (b2 & d2), d1 * 4 + d2 * 2 + d3)
                                 for d1 in range(2) for d2 in range(2)]
                        for t, (sgn, j) in enumerate(terms):
                            nc.tensor.matmul(
                                dst, lhsT=(h128p if sgn == 0 else h128n)[:],
                                rhs=xbj(j), start=(t == 0), stop=(t == 3))
                else:
                    # first H8 stage folded into PE accumulation:
                    # V[2q+b1] = H@x[q] +/- H@x[q+4]  (index layout (d2,d3,b1))
                    for jp in range(8):
                        q, b1 = divmod(jp, 2)
                        dst = (pva if jp < 4 else pvb)[:, jp % 4, :]
                        nc.tensor.matmul(dst, lhsT=h128p[:], rhs=xbj(q),
                                         start=True, stop=False)
                        nc.tensor.matmul(dst,
                                         lhsT=(h128p if b1 == 0 else h128n)[:],
                                         rhs=xbj(q + 4), start=False, stop=True)
                v = vp.tile([P, 8, ASTRIP], bf16, tag="v", name=f"v_{u}")
                nc.scalar.copy(out=v[:, 0:4, :], in_=pva[:])
                nc.scalar.copy(out=v[:, 4:8, :], in_=pvb[:])

                # rotation butterfly: out[2q+0]=in[q]+in[q+4],
                # out[2q+1]=in[q]-in[q+4]: each pass shifts the index bits
                # left, ending at (b1,b2,b3) = natural H8 index
                def stage(dst, src, add_op, sub_op):
                    in0, in1 = src[:, 0:4, :], src[:, 4:8, :]
                    o = dst.rearrange("p (q b) m -> p q b m", b=2)
                    add_op(out=o[:, :, 0, :], in0=in0, in1=in1)
                    sub_op(out=o[:, :, 1, :], in0=in0, in1=in1)

                a = apool.tile([P, 8, ASTRIP], bf16, tag="A", name=f"a_{u}")
                if fold2:
                    # single remaining stage; DVE add + GpSimd sub in parallel
                    stage(a, v, nc.vector.tensor_add, nc.gpsimd.tensor_sub)
                else:
                    s2 = sp.tile([P, 8, ASTRIP], bf16, tag="s", name=f"s2_{u}")
                    stage(s2, v, nc.gpsimd.tensor_add, nc.gpsimd.tensor_sub)
                    stage(a, s2, nc.vector.tensor_add, nc.vector.tensor_sub)
                A[(half, msl, blk)] = a

            def loadw(s, ch):
                half, ns = divmod(s, NS_PER_HALF)
                wst = wstp.tile([P, 2048], f32, tag="wst",
                                name=f"wst_{s}_{ch}")
                nc.sync.dma_start(out=wst[:], in_=wd[ns * 4 + ch])
                wb = wbfp.tile([P, 8, NSTRIP], bf16, tag="wbf",
                               name=f"wb_{s}_{ch}")
                src = wst.rearrange("p (c n) -> p c n", c=8)
                if (s + ch) % 2 == 0:
                    nc.scalar.copy(out=wb[:], in_=src)
                else:
                    nc.vector.tensor_copy(out=wb[:], in_=src)
                WB[(half, ns, ch)] = wb

            def lhsT_of(half, g, kt):
                blk, b = divmod(kt, 8)
                msl, sub = divmod(g, 2)
                return A[(half, msl, blk)][:, b, sub * P:(sub + 1) * P]

            def bmm(pc, half, ns, g, kt):
                nc.tensor.matmul(
                    pc, lhsT=lhsT_of(half, g, kt),
                    rhs=WB[(half, ns, kt // WCH)][:, kt % WCH, :],
                    start=(kt == 0), stop=(kt == 31))

            couts = {}

            def evict_group(half, ns, g, pc, final=False):
                gset, i = divmod(g, 4)
                if i == 0:
                    couts[gset % 2] = outp.tile(
                        [P, 4, NSTRIP], f32, tag="out",
                        name=f"co_{half}_{ns}_{gset}")
                co = couts[gset % 2]
                nc.scalar.copy(out=co[:, i, :], in_=pc)
                ydr = yd[half * 32 + ns * 2 + gset].rearrange(
                    "p (i n) -> p i n", i=4)
                if final:
                    # split the last y store so the tail DMA is shorter
                    if i == 1:
                        nc.sync.dma_start(out=ydr[:, 0:2, :],
                                          in_=co[:, 0:2, :])
                    elif i == 3:
                        nc.sync.dma_start(out=ydr[:, 2:4, :],
                                          in_=co[:, 2:4, :])
                elif i == 3:
                    nc.sync.dma_start(out=ydr, in_=co[:])

            # ---------------- opening strip (half 0, ns 0) ----------------
            aload(0, 0, 0, split=True)
            loadw(0, 0)
            aload(0, 1, 0, split=True)
            loadw(0, 1)
            for gset in range(2):
                msl0, msl1 = (0, 1) if gset == 0 else (2, 3)
                pcs = {}
                for slot in range(5):
                    if slot < 4:
                        acompute(0, msl0, slot, fold2=True)
                        acompute(0, msl1, slot, fold2=True)
                        if slot < 3:
                            aload(0, msl0, slot + 1)
                            aload(0, msl1, slot + 1)
                        elif gset == 0:
                            aload(0, 2, 0)
                            aload(0, 3, 0)
                        if gset == 0 and slot < 2:
                            loadw(0, slot + 2)
                        elif gset == 1:
                            loadw(1, slot)     # strip 1 prefetch
                    if slot > 0:
                        ch = slot - 1
                        for g in (0, 1, 2, 3) if gset == 0 else (4, 5, 6, 7):
                            if ch == 0 and g % 2 == 0:
                                pcs[g // 2] = psC.tile(
                                    [P, 2, NSTRIP], f32, tag="C",
                                    name=f"pc_0_0_{g}")
                            for c8 in range(WCH):
                                bmm(pcs[g // 2][:, g % 2, :], 0, 0, g,
                                    ch * WCH + c8)
                for g in (0, 1, 2, 3) if gset == 0 else (4, 5, 6, 7):
                    evict_group(0, 0, g, pcs[g // 2][:, g % 2, :])

            # ---------------- steady strips ----------------
            # A(1) weave: units #1..16 in B-consumption order
            a1_units = []
            for mpair in ((0, 1), (2, 3)):
                for blk in range(NBLK):
                    a1_units.append((mpair[0], blk))
                    a1_units.append((mpair[1], blk))
            weave = {}   # (s, g) -> list of thunks

            def add_weave(s, g, fn):
                weave.setdefault((s, g), []).append(fn)

            # loads s=12..14 at even groups, computes at odd groups (#1..12)
            for k in range(12):
                s = 12 + k // 4
                g = (k % 4) * 2
                msl, blk = a1_units[k]
                add_weave(s, g, (lambda m, b: lambda: aload(1, m, b))(msl, blk))
                add_weave(s, g + 1,
                          (lambda m, b: lambda: acompute(1, m, b))(msl, blk))
            # units #13..16: load late in s=15, compute at s=16 g0..g3
            for k in range(12, 16):
                msl, blk = a1_units[k]
                add_weave(15, k - 8,
                          (lambda m, b: lambda: aload(1, m, b))(msl, blk))
                add_weave(16, k - 12,
                          (lambda m, b: lambda: acompute(1, m, b))(msl, blk))

            pcpair = [None]
            for s in range(1, 32):
                half, ns = divmod(s, NS_PER_HALF)
                for g in range(8):
                    if s < 31 and g % 2 == 0:
                        loadw(s + 1, g // 2)
                    for fn in weave.get((s, g), ()):
                        fn()
                    if g % 2 == 0:
                        pcpair[0] = psC.tile([P, 2, NSTRIP], f32, tag="C",
                                             name=f"pc_{half}_{ns}_{g}")
                    pc = pcpair[0][:, g % 2, :]
                    for kt in range(32):
                        bmm(pc, half, ns, g, kt)
                    evict_group(half, ns, g, pc, final=(s == 31))

    nc.compile()
    return nc


_CACHE = {}


def _get_nc():
    if "nc" not in _CACHE:
        _CACHE["nc"] = build_nc()
    return _CACHE["nc"]


def _prep_x(xc):
    """[2048, 4096] f32 -> [32, 128, 2048]: unit (ms, blk), [p, j, m]."""
    return np.ascontiguousarray(
        xc.reshape(8, 256, 4, 8, 128).transpose(0, 2, 4, 3, 1)
    ).reshape(32, 128, 2048)


def _prep_w(w):
    """[4096, 4096] (n,k) f32 -> [64, 128, 2048]: chunk (ns, ch), [p, c, n]."""
    return np.ascontiguousarray(
        w.reshape(16, 256, 4, 8, 128).transpose(0, 2, 4, 3, 1)
    ).reshape(64, 128, 2048)


def _unprep_y(ydv):
    """[64, 128, 1024] f32 -> [2048, 4096]."""
    return np.ascontiguousarray(
        ydv.reshape(2, 16, 2, 128, 4, 256).transpose(0, 2, 4, 3, 1, 5)
    ).reshape(2048, 4096)


def run(x, weight, trace=False):
    assert x.shape == (B_FULL, S_FULL, D) and weight.shape == (D, D)
    nc = _get_nc()
    xf = np.asarray(x, dtype=np.float32).reshape(M_FULL, D)
    wv = np.asarray(weight, dtype=np.float32)
    wdv = _prep_w(wv)
    h1 = np.asarray(_h128_np())
    hh = np.concatenate([h1, -h1], axis=0)
    in_maps = [
        {"xd": _prep_x(xf[c * M_CORE:(c + 1) * M_CORE]),
         "wd": wdv, "h": hh}
        for c in range(N_CORES)
    ]
    res = run_bass_kernel_spmd(nc, in_maps, core_ids=list(range(N_CORES)),
                               trace=trace)
    yv = np.concatenate([_unprep_y(r["yd"]) for r in res.results], axis=0)
    return yv.reshape(B_FULL, S_FULL, D), res


def kernel(x, weight):
    return run(x, weight)[0]


# revision 16
# speedup vs baseline: 1.1835x; 1.0166x over previous
"""HadLinear TRN2 kernel: out = fwht_1024blocks(x)/sqrt(1024) @ W.T

Math: fwht on 1024-blocks is x @ H_bd, H_bd = blockdiag(H_1024 x4),
H_1024 = H_8 (x) H_128 (natural order, k = j*128 + p). The 1/sqrt(1024)
= 2^-5 scale is folded into H_128 (exact in bf16).

Sharding: data-parallel row shard of x (2048 rows/core). Host passes all
tensors in device-friendly permuted layouts (pure layout changes) so
every DMA is a fully contiguous >=0.5MB block transfer.

Per core (M_CORE=2048 rows split in 2 halves of 1024):
  Phase A (per unit = 256-row m-strip x one 1024-k-block): DVE casts x to
  bf16; PE computes V[2q+b] = H128@x[q] +/- H128@x[q+4] (first H8
  butterfly folded into PSUM accumulation via +/-H128 constants); ACT
  evicts PSUM to a bf16 [128,8,256] tile; remaining two H8 stages run as
  4 large strided add/sub ops (GpSimd stage, then DVE stage) using a
  bit-rotation layout so every stage is the same op shape.
  Phase B: per 128-row output group g, C[g] = sum_kt A[kt].T @ W[kt,n]
  accumulated k-contiguously in ONE PSUM bank (32 matmuls, N=256), ACT
  evicts to an SBUF staging tile, 0.5MB y DMA per 4 groups.

Schedule (emission order == per-engine issue order): the opening strip
interleaves phase-A units with chunk-wise (kt-outer) phase-B matmuls so
PE work starts as soon as the first x/W bytes land; every later strip is
k-contiguous per group with the next strip's W chunks and the next
half's A units woven between groups, so the PE never waits on phase A
or W loads at steady state.

Self-contained: hardcodes B=4, S=4096, D_in=D_out=4096, 8 cores.
"""

import numpy as np
import ml_dtypes

import concourse.bacc as bacc
import concourse.mybir as mybir
import concourse.tile as tile
from concourse.bass_utils import run_bass_kernel_spmd

P = 128
N_CORES = 8
B_FULL, S_FULL, D = 4, 4096, 4096
M_FULL = B_FULL * S_FULL          # 16384 rows total
M_CORE = M_FULL // N_CORES        # 2048 rows per core
HAD = 1024                        # hadamard block
NBLK = D // HAD                   # 4 k-blocks of 1024
ASTRIP = 256                      # phase A m-strip width
NSTRIP = 256                      # phase B out-feature strip width
NS_PER_HALF = 16                  # strips per half (4096/256)
MS_PER_HALF = 4                   # 256-row m-strips per half
WCH = 8                           # k-tiles per W chunk -> 4 chunks/strip


def _h128_np():
    """H_128 (natural order) scaled by 1/sqrt(1024) = 2^-5; exact in bf16."""
    h = np.array([[(-1.0) ** bin(i & j).count("1") for j in range(P)]
                  for i in range(P)])
    return (h / 32.0).astype(ml_dtypes.bfloat16)


def build_nc():
    f32, bf16 = mybir.dt.float32, mybir.dt.bfloat16
    nc = bacc.Bacc(None, target_bir_lowering=False, debug=False)

    # x: 32 units of [P, 8j, 256m]; unit u = (half*4+msl)*4 + blk
    xd = nc.declare_dram_parameter("xd", [32, P, 2048], f32, isOutput=False)
    # W: 64 chunks of [P, 8c, 256n]; chunk = ns*4 + ch (kt = ch*8 + c)
    wd = nc.declare_dram_parameter("wd", [64, P, 2048], f32, isOutput=False)
    h = nc.declare_dram_parameter("h", [2 * P, P], bf16, isOutput=False)
    # y: 64 blocks of [P, 4i, 256n]; block = half*32 + ns*2 + gset
    yd = nc.declare_dram_parameter("yd", [64, P, 1024], f32, isOutput=True)

    with tile.TileContext(nc) as tc:
        with (
            tc.tile_pool(name="const", bufs=1) as constp,
            tc.tile_pool(name="xs", bufs=3) as xsp,
            tc.tile_pool(name="xb", bufs=2) as xbp,
            tc.tile_pool(name="vv", bufs=2) as vp,
            tc.tile_pool(name="ss", bufs=3) as sp,
            tc.tile_pool(name="apool", bufs=24) as apool,
            tc.tile_pool(name="wst", bufs=2) as wstp,
            tc.tile_pool(name="wbf", bufs=8) as wbfp,
            tc.tile_pool(name="outp", bufs=2) as outp,
            tc.tile_pool(name="psV", bufs=2, space="PSUM") as psV,
            tc.tile_pool(name="psC", bufs=4, space="PSUM") as psC,
        ):
            h128p = constp.tile([P, P], bf16, tag="hp", name="h128p")
            nc.sync.dma_start(out=h128p[:], in_=h[0:P, :])
            h128n = constp.tile([P, P], bf16, tag="hn", name="h128n")
            nc.sync.dma_start(out=h128n[:], in_=h[P:2 * P, :])

            # HAM warm-up: ~3.5us of tiny matmuls during the initial DMA
            # window so the PE clock is at 8/8 when real work arrives.
            warm = psV.tile([P, 4, ASTRIP], f32, tag="V", name="warm")
            for w in range(56):
                nc.tensor.matmul(warm[:, 0, 0:64], lhsT=h128p[:],
                                 rhs=h128p[:, 0:64], start=True, stop=True)

            A = {}      # (half, msl, blk) -> [P, 8b, 256m] bf16
            XB = {}     # staging for loaded-but-not-computed units
            WB = {}     # (half, ns, ch) -> [P, 8c, 256n] bf16

            def aload(half, msl, blk, split=False):
                u = (half * MS_PER_HALF + msl) * NBLK + blk
                xs = xsp.tile([P, 2048], f32, tag="xs", name=f"xs_{u}")
                xb = xbp.tile([P, 2048], bf16, tag="xb", name=f"xb_{u}")
                if split:
                    # halve time-to-first-matmul at kernel start
                    nc.sync.dma_start(out=xs[:, 0:1024], in_=xd[u][:, 0:1024])
                    nc.vector.tensor_copy(out=xb[:, 0:1024],
                                          in_=xs[:, 0:1024])
                    nc.sync.dma_start(out=xs[:, 1024:2048],
                                      in_=xd[u][:, 1024:2048])
                    nc.vector.tensor_copy(out=xb[:, 1024:2048],
                                          in_=xs[:, 1024:2048])
                else:
                    nc.sync.dma_start(out=xs[:], in_=xd[u])
                    nc.vector.tensor_copy(out=xb[:], in_=xs[:])
                XB[(half, msl, blk)] = xb

            def acompute(half, msl, blk, fold2=False):
                u = (half * MS_PER_HALF + msl) * NBLK + blk
                xb = XB.pop((half, msl, blk))
                xbj = lambda j: xb[:, j * ASTRIP:(j + 1) * ASTRIP]
                pva = psV.tile([P, 4, ASTRIP], f32, tag="V", name=f"pva_{u}")
                pvb = psV.tile([P, 4, ASTRIP], f32, tag="V", name=f"pvb_{u}")
                if fold2:
                    # first TWO H8 stages folded into PE accumulation:
                    # V[d3*4+b1*2+b2] = sum_{d1,d2} +/- H@x[d1*4+d2*2+d3]
                    for jp in range(8):
                        d3, r = divmod(jp, 4)
                        b1, b2 = divmod(r, 2)
                        dst = (pva if jp < 4 else pvb)[:, jp % 4, :]
                        terms = [((b1 & d1) ^ (b2 & d2), d1 * 4 + d2 * 2 + d3)
                                 for d1 in range(2) for d2 in range(2)]
                        for t, (sgn, j) in enumerate(terms):
                            nc.tensor.matmul(
                                dst, lhsT=(h128p if sgn == 0 else h128n)[:],
                                rhs=xbj(j), start=(t == 0), stop=(t == 3))
                else:
                    # first H8 stage folded into PE accumulation:
                    # V[2q+b1] = H@x[q] +/- H@x[q+4]  (index layout (d2,d3,b1))
                    for jp in range(8):
                        q, b1 = divmod(jp, 2)
                        dst = (pva if jp < 4 else pvb)[:, jp % 4, :]
                        nc.tensor.matmul(dst, lhsT=h128p[:], rhs=xbj(q),
                                         start=True, stop=False)
                        nc.tensor.matmul(dst,
                                         lhsT=(h128p if b1 == 0 else h128n)[:],
                                         rhs=xbj(q + 4), start=False, stop=True)
                v = vp.tile([P, 8, ASTRIP], bf16, tag="v", name=f"v_{u}")
                nc.scalar.copy(out=v[:, 0:4, :], in_=pva[:])
                nc.scalar.copy(out=v[:, 4:8, :], in_=pvb[:])

                # rotation butterfly: out[2q+0]=in[q]+in[q+4],
                # out[2q+1]=in[q]-in[q+4]: each pass shifts the index bits
                # left, ending at (b1,b2,b3) = natural H8 index
                def stage(dst, src, add_op, sub_op):
                    in0, in1 = src[:, 0:4, :], src[:, 4:8, :]
                    o = dst.rearrange("p (q b) m -> p q b m", b=2)
                    add_op(out=o[:, :, 0, :], in0=in0, in1=in1)
                    sub_op(out=o[:, :, 1, :], in0=in0, in1=in1)

                a = apool.tile([P, 8, ASTRIP], bf16, tag="A", name=f"a_{u}")
                if fold2:
                    # single remaining stage; DVE add + GpSimd sub in parallel
                    stage(a, v, nc.vector.tensor_add, nc.gpsimd.tensor_sub)
                else:
                    s2 = sp.tile([P, 8, ASTRIP], bf16, tag="s", name=f"s2_{u}")
                    stage(s2, v, nc.gpsimd.tensor_add, nc.gpsimd.tensor_sub)
                    stage(a, s2, nc.vector.tensor_add, nc.vector.tensor_sub)
                A[(half, msl, blk)] = a

            def loadw(s, ch):
                half, ns = divmod(s, NS_PER_HALF)
                wst = wstp.tile([P, 2048], f32, tag="wst",
                                name=f"wst_{s}_{ch}")
                nc.sync.dma_start(out=wst[:], in_=wd[ns * 4 + ch])
                wb = wbfp.tile([P, 8, NSTRIP], bf16, tag="wbf",
                               name=f"wb_{s}_{ch}")
                src = wst.rearrange("p (c n) -> p c n", c=8)
                if (s + ch) % 2 == 0:
                    nc.scalar.copy(out=wb[:], in_=src)
                else:
                    nc.vector.tensor_copy(out=wb[:], in_=src)
                WB[(half, ns, ch)] = wb

            def lhsT_of(half, g, kt):
                blk, b = divmod(kt, 8)
                msl, sub = divmod(g, 2)
                return A[(half, msl, blk)][:, b, sub * P:(sub + 1) * P]

            def bmm(pc, half, ns, g, kt):
                nc.tensor.matmul(
                    pc, lhsT=lhsT_of(half, g, kt),
                    rhs=WB[(half, ns, kt // WCH)][:, kt % WCH, :],
                    start=(kt == 0), stop=(kt == 31))

            couts = {}

            def evict_group(half, ns, g, pc, final=False):
                gset, i = divmod(g, 4)
                if i == 0:
                    couts[gset % 2] = outp.tile(
                        [P, 4, NSTRIP], f32, tag="out",
                        name=f"co_{half}_{ns}_{gset}")
                co = couts[gset % 2]
                nc.scalar.copy(out=co[:, i, :], in_=pc)
                ydr = yd[half * 32 + ns * 2 + gset].rearrange(
                    "p (i n) -> p i n", i=4)
                if final:
                    # split the last y store so the tail DMA is shorter
                    if i == 1:
                        nc.sync.dma_start(out=ydr[:, 0:2, :],
                                          in_=co[:, 0:2, :])
                    elif i == 3:
                        nc.sync.dma_start(out=ydr[:, 2:4, :],
                                          in_=co[:, 2:4, :])
                elif i == 3:
                    nc.sync.dma_start(out=ydr, in_=co[:])

            # ---------------- opening strip (half 0, ns 0) ----------------
            aload(0, 0, 0, split=True)
            loadw(0, 0)
            aload(0, 1, 0, split=True)
            loadw(0, 1)
            for gset in range(2):
                msl0, msl1 = (0, 1) if gset == 0 else (2, 3)
                pcs = {}
                for slot in range(5):
                    if slot < 4:
                        acompute(0, msl0, slot, fold2=True)
                        acompute(0, msl1, slot, fold2=True)
                        if slot < 3:
                            aload(0, msl0, slot + 1)
                            aload(0, msl1, slot + 1)
                        elif gset == 0:
                            aload(0, 2, 0)
                            aload(0, 3, 0)
                        if gset == 0 and slot < 2:
                            loadw(0, slot + 2)
                        elif gset == 1:
                            loadw(1, slot)     # strip 1 prefetch
                    if slot > 0:
                        ch = slot - 1
                        for g in (0, 1, 2, 3) if gset == 0 else (4, 5, 6, 7):
                            if ch == 0:
                                pcs[g] = psC.tile([P, NSTRIP], f32, tag="C",
                                                  name=f"pc_0_0_{g}")
                            for c8 in range(WCH):
                                bmm(pcs[g][:], 0, 0, g, ch * WCH + c8)
                for g in (0, 1, 2, 3) if gset == 0 else (4, 5, 6, 7):
                    evict_group(0, 0, g, pcs[g][:])

            # ---------------- steady strips ----------------
            # A(1) weave: units #1..16 in B-consumption order
            a1_units = []
            for mpair in ((0, 1), (2, 3)):
                for blk in range(NBLK):
                    a1_units.append((mpair[0], blk))
                    a1_units.append((mpair[1], blk))
            weave = {}   # (s, g) -> list of thunks

            def add_weave(s, g, fn):
                weave.setdefault((s, g), []).append(fn)

            # loads s=12..14 at even groups, computes at odd groups (#1..12)
            for k in range(12):
                s = 12 + k // 4
                g = (k % 4) * 2
                msl, blk = a1_units[k]
                add_weave(s, g, (lambda m, b: lambda: aload(1, m, b))(msl, blk))
                add_weave(s, g + 1,
                          (lambda m, b: lambda: acompute(1, m, b))(msl, blk))
            # units #13..16: load late in s=15, compute at s=16 g0..g3
            for k in range(12, 16):
                msl, blk = a1_units[k]
                add_weave(15, k - 8,
                          (lambda m, b: lambda: aload(1, m, b))(msl, blk))
                add_weave(16, k - 12,
                          (lambda m, b: lambda: acompute(1, m, b))(msl, blk))

            for s in range(1, 32):
                half, ns = divmod(s, NS_PER_HALF)
                for g in range(8):
                    if s < 31 and g % 2 == 0:
                        loadw(s + 1, g // 2)
                    for fn in weave.get((s, g), ()):
                        fn()
                    pc = psC.tile([P, NSTRIP], f32, tag="C",
                                  name=f"pc_{half}_{ns}_{g}")
                    for kt in range(32):
                        bmm(pc[:], half, ns, g, kt)
                    evict_group(half, ns, g, pc[:], final=(s == 31))

    nc.compile()
    return nc


_CACHE = {}


def _get_nc():
    if "nc" not in _CACHE:
        _CACHE["nc"] = build_nc()
    return _CACHE["nc"]


def _prep_x(xc):
    """[2048, 4096] f32 -> [32, 128, 2048]: unit (ms, blk), [p, j, m]."""
    return np.ascontiguousarray(
        xc.reshape(8, 256, 4, 8, 128).transpose(0, 2, 4, 3, 1)
    ).reshape(32, 128, 2048)


def _prep_w(w):
    """[4096, 4096] (n,k) f32 -> [64, 128, 2048]: chunk (ns, ch), [p, c, n]."""
    return np.ascontiguousarray(
        w.reshape(16, 256, 4, 8, 128).transpose(0, 2, 4, 3, 1)
    ).reshape(64, 128, 2048)


def _unprep_y(ydv):
    """[64, 128, 1024] f32 -> [2048, 4096]."""
    return np.ascontiguousarray(
        ydv.reshape(2, 16, 2, 128, 4, 256).transpose(0, 2, 4, 3, 1, 5)
    ).reshape(2048, 4096)


def run(x, weight, trace=False):
    assert x.shape == (B_FULL, S_FULL, D) and weight.shape == (D, D)
    nc = _get_nc()
    xf = np.asarray(x, dtype=np.float32).reshape(M_FULL, D)
    wv = np.asarray(weight, dtype=np.float32)
    wdv = _prep_w(wv)
    h1 = np.asarray(_h128_np())
    hh = np.concatenate([h1, -h1], axis=0)
    in_maps = [
        {"xd": _prep_x(xf[c * M_CORE:(c + 1) * M_CORE]),
         "wd": wdv, "h": hh}
        for c in range(N_CORES)
    ]
    res = run_bass_kernel_spmd(nc, in_maps, core_ids=list(range(N_CORES)),
                               trace=trace)
    yv = np.concatenate([_unprep_y(r["yd"]) for r in res.results], axis=0)
    return yv.reshape(B_FULL, S_FULL, D), res


def kernel(x, weight):
    return run(x, weight)[0]


# revision 22
# speedup vs baseline: 1.1842x; 1.0006x over previous
"""HadLinear TRN2 kernel: out = fwht_1024blocks(x)/sqrt(1024) @ W.T

Math: fwht on 1024-blocks is x @ H_bd, H_bd = blockdiag(H_1024 x4),
H_1024 = H_8 (x) H_128 (natural order, k = j*128 + p). The 1/sqrt(1024)
= 2^-5 scale is folded into H_128 (exact in bf16).

Sharding: data-parallel row shard of x (2048 rows/core). Host passes all
tensors in device-friendly permuted layouts (pure layout changes) so
every DMA is a fully contiguous >=0.5MB block transfer.

Per core (M_CORE=2048 rows split in 2 halves of 1024):
  Phase A (per unit = 256-row m-strip x one 1024-k-block): DVE casts x to
  bf16; PE computes V[2q+b] = H128@x[q] +/- H128@x[q+4] (first H8
  butterfly folded into PSUM accumulation via +/-H128 constants); ACT
  evicts PSUM to a bf16 [128,8,256] tile; remaining two H8 stages run as
  4 large strided add/sub ops (GpSimd stage, then DVE stage) using a
  bit-rotation layout so every stage is the same op shape.
  Phase B: per 128-row output group g, C[g] = sum_kt A[kt].T @ W[kt,n]
  accumulated k-contiguously in ONE PSUM bank (32 matmuls, N=256), ACT
  evicts to an SBUF staging tile, 0.5MB y DMA per 4 groups.

Schedule (emission order == per-engine issue order): the opening strip
interleaves phase-A units with chunk-wise (kt-outer) phase-B matmuls so
PE work starts as soon as the first x/W bytes land; every later strip is
k-contiguous per group with the next strip's W chunks and the next
half's A units woven between groups, so the PE never waits on phase A
or W loads at steady state.

Self-contained: hardcodes B=4, S=4096, D_in=D_out=4096, 8 cores.
"""

import numpy as np
import ml_dtypes

import concourse.bacc as bacc
import concourse.mybir as mybir
import concourse.tile as tile
from concourse.bass_utils import run_bass_kernel_spmd

P = 128
N_CORES = 8
B_FULL, S_FULL, D = 4, 4096, 4096
M_FULL = B_FULL * S_FULL          # 16384 rows total
M_CORE = M_FULL // N_CORES        # 2048 rows per core
HAD = 1024                        # hadamard block
NBLK = D // HAD                   # 4 k-blocks of 1024
ASTRIP = 256                      # phase A m-strip width
NSTRIP = 256                      # phase B out-feature strip width
NS_PER_HALF = 16                  # strips per half (4096/256)
MS_PER_HALF = 4                   # 256-row m-strips per half
WCH = 8                           # k-tiles per W chunk -> 4 chunks/strip


def _h128_np():
    """H_128 (natural order) scaled by 1/sqrt(1024) = 2^-5; exact in bf16."""
    h = np.array([[(-1.0) ** bin(i & j).count("1") for j in range(P)]
                  for i in range(P)])
    return (h / 32.0).astype(ml_dtypes.bfloat16)


def build_nc():
    f32, bf16 = mybir.dt.float32, mybir.dt.bfloat16
    nc = bacc.Bacc(None, target_bir_lowering=False, debug=False)

    # x: 32 units of [P, 8j, 256m]; unit u = (half*4+msl)*4 + blk
    xd = nc.declare_dram_parameter("xd", [32, P, 2048], f32, isOutput=False)
    # W: 64 chunks of [P, 8c, 256n]; chunk = ns*4 + ch (kt = ch*8 + c)
    wd = nc.declare_dram_parameter("wd", [64, P, 2048], f32, isOutput=False)
    h = nc.declare_dram_parameter("h", [2 * P, P], bf16, isOutput=False)
    # y: 64 blocks of [P, 4i, 256n]; block = half*32 + ns*2 + gset
    yd = nc.declare_dram_parameter("yd", [64, P, 1024], f32, isOutput=True)

    with tile.TileContext(nc) as tc:
        with (
            tc.tile_pool(name="const", bufs=1) as constp,
            tc.tile_pool(name="xs", bufs=3) as xsp,
            tc.tile_pool(name="xb", bufs=2) as xbp,
            tc.tile_pool(name="vv", bufs=2) as vp,
            tc.tile_pool(name="ss", bufs=3) as sp,
            tc.tile_pool(name="apool", bufs=24) as apool,
            tc.tile_pool(name="wbf", bufs=8) as wbfp,
            tc.tile_pool(name="outp", bufs=2) as outp,
            tc.tile_pool(name="psV", bufs=2, space="PSUM") as psV,
            tc.tile_pool(name="psC", bufs=4, space="PSUM") as psC,
        ):
            h128p = constp.tile([P, P], bf16, tag="hp", name="h128p")
            nc.sync.dma_start(out=h128p[:], in_=h[0:P, :])
            h128n = constp.tile([P, P], bf16, tag="hn", name="h128n")
            nc.sync.dma_start(out=h128n[:], in_=h[P:2 * P, :])

            A = {}      # (half, msl, blk) -> [P, 8b, 256m] bf16
            XB = {}     # staging for loaded-but-not-computed units
            WB = {}     # (half, ns, ch) -> [P, 8c, 256n] bf16

            def aload(half, msl, blk):
                u = (half * MS_PER_HALF + msl) * NBLK + blk
                xs = xsp.tile([P, 2048], f32, tag="xs", name=f"xs_{u}")
                nc.sync.dma_start(out=xs[:], in_=xd[u])
                xb = xbp.tile([P, 2048], bf16, tag="xb", name=f"xb_{u}")
                nc.scalar.copy(out=xb[:], in_=xs[:])   # ACT cast; DVE is busy
                XB[(half, msl, blk)] = xb

            def acompute(half, msl, blk, fold2=False):
                u = (half * MS_PER_HALF + msl) * NBLK + blk
                xb = XB.pop((half, msl, blk))
                xbj = lambda j: xb[:, j * ASTRIP:(j + 1) * ASTRIP]
                pva = psV.tile([P, 4, ASTRIP], f32, tag="V", name=f"pva_{u}")
                pvb = psV.tile([P, 4, ASTRIP], f32, tag="V", name=f"pvb_{u}")
                if fold2:
                    # first TWO H8 stages folded into PE accumulation:
                    # V[d3*4+b1*2+b2] = sum_{d1,d2} +/- H@x[d1*4+d2*2+d3]
                    for jp in range(8):
                        d3, r = divmod(jp, 4)
                        b1, b2 = divmod(r, 2)
                        dst = (pva if jp < 4 else pvb)[:, jp % 4, :]
                        terms = [((b1 & d1) ^ (b2 & d2), d1 * 4 + d2 * 2 + d3)
                                 for d1 in range(2) for d2 in range(2)]
                        for t, (sgn, j) in enumerate(terms):
                            nc.tensor.matmul(
                                dst, lhsT=(h128p if sgn == 0 else h128n)[:],
                                rhs=xbj(j), start=(t == 0), stop=(t == 3))
                else:
                    # first H8 stage folded into PE accumulation:
                    # V[2q+b1] = H@x[q] +/- H@x[q+4]  (index layout (d2,d3,b1))
                    for jp in range(8):
                        q, b1 = divmod(jp, 2)
                        dst = (pva if jp < 4 else pvb)[:, jp % 4, :]
                        nc.tensor.matmul(dst, lhsT=h128p[:], rhs=xbj(q),
                                         start=True, stop=False)
                        nc.tensor.matmul(dst,
                                         lhsT=(h128p if b1 == 0 else h128n)[:],
                                         rhs=xbj(q + 4), start=False, stop=True)
                v = vp.tile([P, 8, ASTRIP], bf16, tag="v", name=f"v_{u}")
                nc.scalar.copy(out=v[:, 0:4, :], in_=pva[:])
                nc.scalar.copy(out=v[:, 4:8, :], in_=pvb[:])

                # rotation butterfly: out[2q+0]=in[q]+in[q+4],
                # out[2q+1]=in[q]-in[q+4]: each pass shifts the index bits
                # left, ending at (b1,b2,b3) = natural H8 index
                def stage(dst, src, add_op, sub_op):
                    in0, in1 = src[:, 0:4, :], src[:, 4:8, :]
                    o = dst.rearrange("p (q b) m -> p q b m", b=2)
                    add_op(out=o[:, :, 0, :], in0=in0, in1=in1)
                    sub_op(out=o[:, :, 1, :], in0=in0, in1=in1)

                a = apool.tile([P, 8, ASTRIP], bf16, tag="A", name=f"a_{u}")
                if fold2:
                    # single remaining stage; DVE add + GpSimd sub in parallel
                    stage(a, v, nc.vector.tensor_add, nc.gpsimd.tensor_sub)
                else:
                    # 3 of 4 stage ops on DVE (2x faster than GpSimd here)
                    s2 = sp.tile([P, 8, ASTRIP], bf16, tag="s", name=f"s2_{u}")
                    stage(s2, v, nc.vector.tensor_add, nc.gpsimd.tensor_sub)
                    stage(a, s2, nc.vector.tensor_add, nc.vector.tensor_sub)
                A[(half, msl, blk)] = a

            def loadw(s, ch):
                half, ns = divmod(s, NS_PER_HALF)
                wb = wbfp.tile([P, 8, NSTRIP], bf16, tag="wbf",
                               name=f"wb_{s}_{ch}")
                nc.gpsimd.dma_start(                    # casting DMA (SWDGE)
                    out=wb[:],
                    in_=wd[ns * 4 + ch].rearrange("p (c n) -> p c n", c=8))
                WB[(half, ns, ch)] = wb

            def lhsT_of(half, g, kt):
                blk, b = divmod(kt, 8)
                msl, sub = divmod(g, 2)
                return A[(half, msl, blk)][:, b, sub * P:(sub + 1) * P]

            def bmm(pc, half, ns, g, kt):
                nc.tensor.matmul(
                    pc[:], lhsT=lhsT_of(half, g, kt),
                    rhs=WB[(half, ns, kt // WCH)][:, kt % WCH, :],
                    start=(kt == 0), stop=(kt == 31))

            couts = {}

            def evict_group(half, ns, g, pc):
                gset, i = divmod(g, 4)
                if i == 0:
                    couts[gset % 2] = outp.tile(
                        [P, 4, NSTRIP], f32, tag="out",
                        name=f"co_{half}_{ns}_{gset}")
                co = couts[gset % 2]
                nc.scalar.copy(out=co[:, i, :], in_=pc[:])
                if i == 3:
                    nc.sync.dma_start(
                        out=yd[half * 32 + ns * 2 + gset].rearrange(
                            "p (i n) -> p i n", i=4),
                        in_=co[:])

            # ---------------- opening strip (half 0, ns 0) ----------------
            aload(0, 0, 0)
            loadw(0, 0)
            aload(0, 1, 0)
            loadw(0, 1)
            for gset in range(2):
                msl0, msl1 = (0, 1) if gset == 0 else (2, 3)
                pcs = {}
                for slot in range(6):
                    if slot < 4:
                        # fold1: vector engines keep pace now that W needs
                        # no cast ops and x casts ride on ACT
                        acompute(0, msl0, slot)
                        acompute(0, msl1, slot)
                        if slot < 3:
                            aload(0, msl0, slot + 1)
                            aload(0, msl1, slot + 1)
                        elif gset == 0:
                            aload(0, 2, 0)
                            aload(0, 3, 0)
                        if gset == 0 and slot < 2:
                            loadw(0, slot + 2)
                        elif gset == 1:
                            loadw(1, slot)     # strip 1 prefetch
                    if slot >= 2:
                        # B lags A by 2 slots to cover the butterfly chain
                        ch = slot - 2
                        for g in (0, 1, 2, 3) if gset == 0 else (4, 5, 6, 7):
                            if ch == 0:
                                pcs[g] = psC.tile([P, NSTRIP], f32, tag="C",
                                                  name=f"pc_0_0_{g}")
                            for c8 in range(WCH):
                                bmm(pcs[g], 0, 0, g, ch * WCH + c8)
                for g in (0, 1, 2, 3) if gset == 0 else (4, 5, 6, 7):
                    evict_group(0, 0, g, pcs[g])

            # ---------------- steady strips ----------------
            # A(1) weave: units #1..16 in B-consumption order
            a1_units = []
            for mpair in ((0, 1), (2, 3)):
                for blk in range(NBLK):
                    a1_units.append((mpair[0], blk))
                    a1_units.append((mpair[1], blk))
            weave = {}   # (s, g) -> list of thunks

            def add_weave(s, g, fn):
                weave.setdefault((s, g), []).append(fn)

            # loads s=12..14 at even groups, computes at odd groups (#1..12)
            for k in range(12):
                s = 12 + k // 4
                g = (k % 4) * 2
                msl, blk = a1_units[k]
                add_weave(s, g, (lambda m, b: lambda: aload(1, m, b))(msl, blk))
                add_weave(s, g + 1,
                          (lambda m, b: lambda: acompute(1, m, b))(msl, blk))
            # units #13..16: load late in s=15, compute at s=16 g0..g3
            for k in range(12, 16):
                msl, blk = a1_units[k]
                add_weave(15, k - 8,
                          (lambda m, b: lambda: aload(1, m, b))(msl, blk))
                add_weave(16, k - 12,
                          (lambda m, b: lambda: acompute(1, m, b))(msl, blk))

            for s in range(1, 32):
                half, ns = divmod(s, NS_PER_HALF)
                for g in range(8):
                    if s < 31 and g % 2 == 0:
                        loadw(s + 1, g // 2)
                    for fn in weave.get((s, g), ()):
                        fn()
                    pc = psC.tile([P, NSTRIP], f32, tag="C",
                                  name=f"pc_{half}_{ns}_{g}")
                    for kt in range(32):
                        bmm(pc, half, ns, g, kt)
                    evict_group(half, ns, g, pc)

    nc.compile()
    return nc


_CACHE = {}


def _get_nc():
    if "nc" not in _CACHE:
        _CACHE["nc"] = build_nc()
    return _CACHE["nc"]


def _prep_x(xc):
    """[2048, 4096] f32 -> [32, 128, 2048]: unit (ms, blk), [p, j, m]."""
    return np.ascontiguousarray(
        xc.reshape(8, 256, 4, 8, 128).transpose(0, 2, 4, 3, 1)
    ).reshape(32, 128, 2048)


def _prep_w(w):
    """[4096, 4096] (n,k) f32 -> [64, 128, 2048]: chunk (ns, ch), [p, c, n]."""
    return np.ascontiguousarray(
        w.reshape(16, 256, 4, 8, 128).transpose(0, 2, 4, 3, 1)
    ).reshape(64, 128, 2048)


def _unprep_y(ydv):
    """[64, 128, 1024] f32 -> [2048, 4096]."""
    return np.ascontiguousarray(
        ydv.reshape(2, 16, 2, 128, 4, 256).transpose(0, 2, 4, 3, 1, 5)
    ).reshape(2048, 4096)


def run(x, weight, trace=False):
    assert x.shape == (B_FULL, S_FULL, D) and weight.shape == (D, D)
    nc = _get_nc()
    xf = np.asarray(x, dtype=np.float32).reshape(M_FULL, D)
    wv = np.asarray(weight, dtype=np.float32)
    wdv = _prep_w(wv)
    h1 = np.asarray(_h128_np())
    hh = np.concatenate([h1, -h1], axis=0)
    in_maps = [
        {"xd": _prep_x(xf[c * M_CORE:(c + 1) * M_CORE]),
         "wd": wdv, "h": hh}
        for c in range(N_CORES)
    ]
    res = run_bass_kernel_spmd(nc, in_maps, core_ids=list(range(N_CORES)),
                               trace=trace)
    yv = np.concatenate([_unprep_y(r["yd"]) for r in res.results], axis=0)
    return yv.reshape(B_FULL, S_FULL, D), res


def kernel(x, weight):
    return run(x, weight)[0]


# revision 24
# speedup vs baseline: 1.1909x; 1.0057x over previous
"""HadLinear TRN2 kernel: out = fwht_1024blocks(x)/sqrt(1024) @ W.T

Math: fwht on 1024-blocks is x @ H_bd, H_bd = blockdiag(H_1024 x4),
H_1024 = H_8 (x) H_128 (natural order, k = j*128 + p). The 1/sqrt(1024)
= 2^-5 scale is folded into H_128 (exact in bf16).

Sharding: data-parallel row shard of x (2048 rows/core). Host passes all
tensors in device-friendly permuted layouts (pure layout changes) so
every DMA is a fully contiguous >=0.5MB block transfer.

Per core (M_CORE=2048 rows split in 2 halves of 1024):
  Phase A (per unit = 256-row m-strip x one 1024-k-block): DVE casts x to
  bf16; PE computes V[2q+b] = H128@x[q] +/- H128@x[q+4] (first H8
  butterfly folded into PSUM accumulation via +/-H128 constants); ACT
  evicts PSUM to a bf16 [128,8,256] tile; remaining two H8 stages run as
  4 large strided add/sub ops (GpSimd stage, then DVE stage) using a
  bit-rotation layout so every stage is the same op shape.
  Phase B: per 128-row output group g, C[g] = sum_kt A[kt].T @ W[kt,n]
  accumulated k-contiguously in ONE PSUM bank (32 matmuls, N=256), ACT
  evicts to an SBUF staging tile, 0.5MB y DMA per 4 groups.

Schedule (emission order == per-engine issue order): the opening strip
interleaves phase-A units with chunk-wise (kt-outer) phase-B matmuls so
PE work starts as soon as the first x/W bytes land; every later strip is
k-contiguous per group with the next strip's W chunks and the next
half's A units woven between groups, so the PE never waits on phase A
or W loads at steady state.

Self-contained: hardcodes B=4, S=4096, D_in=D_out=4096, 8 cores.
"""

import numpy as np
import ml_dtypes

import concourse.bacc as bacc
import concourse.mybir as mybir
import concourse.tile as tile
from concourse.bass_utils import run_bass_kernel_spmd

P = 128
N_CORES = 8
B_FULL, S_FULL, D = 4, 4096, 4096
M_FULL = B_FULL * S_FULL          # 16384 rows total
M_CORE = M_FULL // N_CORES        # 2048 rows per core
HAD = 1024                        # hadamard block
NBLK = D // HAD                   # 4 k-blocks of 1024
ASTRIP = 256                      # phase A m-strip width
NSTRIP = 256                      # phase B out-feature strip width
NS_PER_HALF = 16                  # strips per half (4096/256)
MS_PER_HALF = 4                   # 256-row m-strips per half
WCH = 8                           # k-tiles per W chunk -> 4 chunks/strip


def _h128_np():
    """H_128 (natural order) scaled by 1/sqrt(1024) = 2^-5; exact in bf16."""
    h = np.array([[(-1.0) ** bin(i & j).count("1") for j in range(P)]
                  for i in range(P)])
    return (h / 32.0).astype(ml_dtypes.bfloat16)


def build_nc():
    f32, bf16 = mybir.dt.float32, mybir.dt.bfloat16
    nc = bacc.Bacc(None, target_bir_lowering=False, debug=False)

    # x: 32 units of [P, 8j, 256m]; unit u = (half*4+msl)*4 + blk
    xd = nc.declare_dram_parameter("xd", [32, P, 2048], f32, isOutput=False)
    # W: 64 chunks of [P, 8c, 256n]; chunk = ns*4 + ch (kt = ch*8 + c)
    wd = nc.declare_dram_parameter("wd", [64, P, 2048], f32, isOutput=False)
    h = nc.declare_dram_parameter("h", [2 * P, P], bf16, isOutput=False)
    # y: 64 blocks of [P, 4i, 256n]; block = half*32 + ns*2 + gset
    yd = nc.declare_dram_parameter("yd", [64, P, 1024], f32, isOutput=True)

    with tile.TileContext(nc) as tc:
        with (
            tc.tile_pool(name="const", bufs=1) as constp,
            tc.tile_pool(name="xs", bufs=3) as xsp,
            tc.tile_pool(name="xb", bufs=2) as xbp,
            tc.tile_pool(name="vv", bufs=2) as vp,
            tc.tile_pool(name="ss", bufs=3) as sp,
            tc.tile_pool(name="apool", bufs=24) as apool,
            tc.tile_pool(name="wbf", bufs=8) as wbfp,
            tc.tile_pool(name="outp", bufs=2) as outp,
            tc.tile_pool(name="psV", bufs=2, space="PSUM") as psV,
            tc.tile_pool(name="psC", bufs=4, space="PSUM") as psC,
        ):
            h128p = constp.tile([P, P], bf16, tag="hp", name="h128p")
            nc.sync.dma_start(out=h128p[:], in_=h[0:P, :])
            h128n = constp.tile([P, P], bf16, tag="hn", name="h128n")
            nc.sync.dma_start(out=h128n[:], in_=h[P:2 * P, :])

            A = {}      # (half, msl, blk) -> [P, 8b, 256m] bf16
            XB = {}     # staging for loaded-but-not-computed units
            WB = {}     # (half, ns, ch) -> [P, 8c, 256n] bf16

            def aload(half, msl, blk):
                u = (half * MS_PER_HALF + msl) * NBLK + blk
                xs = xsp.tile([P, 2048], f32, tag="xs", name=f"xs_{u}")
                nc.sync.dma_start(out=xs[:], in_=xd[u])
                xb = xbp.tile([P, 2048], bf16, tag="xb", name=f"xb_{u}")
                nc.scalar.copy(out=xb[:], in_=xs[:])   # ACT cast; DVE is busy
                XB[(half, msl, blk)] = xb

            def acompute(half, msl, blk, fold2=False):
                u = (half * MS_PER_HALF + msl) * NBLK + blk
                xb = XB.pop((half, msl, blk))
                xbj = lambda j: xb[:, j * ASTRIP:(j + 1) * ASTRIP]
                pva = psV.tile([P, 4, ASTRIP], f32, tag="V", name=f"pva_{u}")
                pvb = psV.tile([P, 4, ASTRIP], f32, tag="V", name=f"pvb_{u}")
                if fold2:
                    # first TWO H8 stages folded into PE accumulation:
                    # V[d3*4+b1*2+b2] = sum_{d1,d2} +/- H@x[d1*4+d2*2+d3]
                    for jp in range(8):
                        d3, r = divmod(jp, 4)
                        b1, b2 = divmod(r, 2)
                        dst = (pva if jp < 4 else pvb)[:, jp % 4, :]
                        terms = [((b1 & d1) ^ (b2 & d2), d1 * 4 + d2 * 2 + d3)
                                 for d1 in range(2) for d2 in range(2)]
                        for t, (sgn, j) in enumerate(terms):
                            nc.tensor.matmul(
                                dst, lhsT=(h128p if sgn == 0 else h128n)[:],
                                rhs=xbj(j), start=(t == 0), stop=(t == 3))
                else:
                    # first H8 stage folded into PE accumulation:
                    # V[2q+b1] = H@x[q] +/- H@x[q+4]  (index layout (d2,d3,b1))
                    for jp in range(8):
                        q, b1 = divmod(jp, 2)
                        dst = (pva if jp < 4 else pvb)[:, jp % 4, :]
                        nc.tensor.matmul(dst, lhsT=h128p[:], rhs=xbj(q),
                                         start=True, stop=False)
                        nc.tensor.matmul(dst,
                                         lhsT=(h128p if b1 == 0 else h128n)[:],
                                         rhs=xbj(q + 4), start=False, stop=True)
                v = vp.tile([P, 8, ASTRIP], bf16, tag="v", name=f"v_{u}")
                nc.scalar.copy(out=v[:, 0:4, :], in_=pva[:])
                nc.scalar.copy(out=v[:, 4:8, :], in_=pvb[:])

                # rotation butterfly: out[2q+0]=in[q]+in[q+4],
                # out[2q+1]=in[q]-in[q+4]: each pass shifts the index bits
                # left, ending at (b1,b2,b3) = natural H8 index
                def stage(dst, src, add_op, sub_op):
                    in0, in1 = src[:, 0:4, :], src[:, 4:8, :]
                    o = dst.rearrange("p (q b) m -> p q b m", b=2)
                    add_op(out=o[:, :, 0, :], in0=in0, in1=in1)
                    sub_op(out=o[:, :, 1, :], in0=in0, in1=in1)

                a = apool.tile([P, 8, ASTRIP], bf16, tag="A", name=f"a_{u}")
                if fold2:
                    # single remaining stage; DVE add + GpSimd sub in parallel
                    stage(a, v, nc.vector.tensor_add, nc.gpsimd.tensor_sub)
                else:
                    # 3 of 4 stage ops on DVE (2x faster than GpSimd here)
                    s2 = sp.tile([P, 8, ASTRIP], bf16, tag="s", name=f"s2_{u}")
                    stage(s2, v, nc.vector.tensor_add, nc.gpsimd.tensor_sub)
                    stage(a, s2, nc.vector.tensor_add, nc.vector.tensor_sub)
                A[(half, msl, blk)] = a

            def loadw(s, ch):
                half, ns = divmod(s, NS_PER_HALF)
                wb = wbfp.tile([P, 8, NSTRIP], bf16, tag="wbf",
                               name=f"wb_{s}_{ch}")
                nc.gpsimd.dma_start(                    # casting DMA (SWDGE)
                    out=wb[:],
                    in_=wd[ns * 4 + ch].rearrange("p (c n) -> p c n", c=8))
                WB[(half, ns, ch)] = wb

            def lhsT_of(half, g, kt):
                blk, b = divmod(kt, 8)
                msl, sub = divmod(g, 2)
                return A[(half, msl, blk)][:, b, sub * P:(sub + 1) * P]

            def bmm(pc, half, ns, g, kt):
                nc.tensor.matmul(
                    pc[:], lhsT=lhsT_of(half, g, kt),
                    rhs=WB[(half, ns, kt // WCH)][:, kt % WCH, :],
                    start=(kt == 0), stop=(kt == 31))

            couts = {}

            def evict_group(half, ns, g, pc):
                gset, i = divmod(g, 4)
                if i == 0:
                    couts[gset % 2] = outp.tile(
                        [P, 4, NSTRIP], f32, tag="out",
                        name=f"co_{half}_{ns}_{gset}")
                co = couts[gset % 2]
                nc.scalar.copy(out=co[:, i, :], in_=pc[:])
                if i == 3:
                    nc.sync.dma_start(
                        out=yd[half * 32 + ns * 2 + gset].rearrange(
                            "p (i n) -> p i n", i=4),
                        in_=co[:])

            # ---------------- opening strip (half 0, ns 0) ----------------
            aload(0, 0, 0)
            loadw(0, 0)
            aload(0, 1, 0)
            loadw(0, 1)
            for gset in range(2):
                msl0, msl1 = (0, 1) if gset == 0 else (2, 3)
                pcs = {}
                for slot in range(5):
                    if slot < 4:
                        # fold1: vector engines keep pace now that W needs
                        # no cast ops and x casts ride on ACT
                        acompute(0, msl0, slot)
                        acompute(0, msl1, slot)
                        if slot < 3:
                            aload(0, msl0, slot + 1)
                            aload(0, msl1, slot + 1)
                        elif gset == 0:
                            aload(0, 2, 0)
                            aload(0, 3, 0)
                        if gset == 0 and slot < 2:
                            loadw(0, slot + 2)
                        elif gset == 1:
                            loadw(1, slot)     # strip 1 prefetch
                    if slot >= 1:
                        # B lags A by 1 slot; fills PE while chains drain
                        ch = slot - 1
                        for g in (0, 1, 2, 3) if gset == 0 else (4, 5, 6, 7):
                            if ch == 0:
                                pcs[g] = psC.tile([P, NSTRIP], f32, tag="C",
                                                  name=f"pc_0_0_{g}")
                            for c8 in range(WCH):
                                bmm(pcs[g], 0, 0, g, ch * WCH + c8)
                for g in (0, 1, 2, 3) if gset == 0 else (4, 5, 6, 7):
                    evict_group(0, 0, g, pcs[g])

            # ---------------- steady strips ----------------
            # A(1) weave: units #1..16 in B-consumption order
            a1_units = []
            for mpair in ((0, 1), (2, 3)):
                for blk in range(NBLK):
                    a1_units.append((mpair[0], blk))
                    a1_units.append((mpair[1], blk))
            weave = {}   # (s, g) -> list of thunks

            def add_weave(s, g, fn):
                weave.setdefault((s, g), []).append(fn)

            # loads s=12..14 at even groups, computes at odd groups (#1..12)
            for k in range(12):
                s = 12 + k // 4
                g = (k % 4) * 2
                msl, blk = a1_units[k]
                add_weave(s, g, (lambda m, b: lambda: aload(1, m, b))(msl, blk))
                add_weave(s, g + 1,
                          (lambda m, b: lambda: acompute(1, m, b))(msl, blk))
            # units #13..16: load late in s=15, compute at s=16 g0..g3
            for k in range(12, 16):
                msl, blk = a1_units[k]
                add_weave(15, k - 8,
                          (lambda m, b: lambda: aload(1, m, b))(msl, blk))
                add_weave(16, k - 12,
                          (lambda m, b: lambda: acompute(1, m, b))(msl, blk))

            for s in range(1, 32):
                half, ns = divmod(s, NS_PER_HALF)
                for g in range(8):
                    if s < 31 and g % 2 == 0:
                        loadw(s + 1, g // 2)
                    for fn in weave.get((s, g), ()):
                        fn()
                    pc = psC.tile([P, NSTRIP], f32, tag="C",
                                  name=f"pc_{half}_{ns}_{g}")
                    for kt in range(32):
                        bmm(pc, half, ns, g, kt)
                    evict_group(half, ns, g, pc)

    nc.compile()
    return nc


_CACHE = {}


def _get_nc():
    if "nc" not in _CACHE:
        _CACHE["nc"] = build_nc()
    return _CACHE["nc"]


def _prep_x(xc):
    """[2048, 4096] f32 -> [32, 128, 2048]: unit (ms, blk), [p, j, m]."""
    return np.ascontiguousarray(
        xc.reshape(8, 256, 4, 8, 128).transpose(0, 2, 4, 3, 1)
    ).reshape(32, 128, 2048)


def _prep_w(w):
    """[4096, 4096] (n,k) f32 -> [64, 128, 2048]: chunk (ns, ch), [p, c, n]."""
    return np.ascontiguousarray(
        w.reshape(16, 256, 4, 8, 128).transpose(0, 2, 4, 3, 1)
    ).reshape(64, 128, 2048)


def _unprep_y(ydv):
    """[64, 128, 1024] f32 -> [2048, 4096]."""
    return np.ascontiguousarray(
        ydv.reshape(2, 16, 2, 128, 4, 256).transpose(0, 2, 4, 3, 1, 5)
    ).reshape(2048, 4096)


def run(x, weight, trace=False):
    assert x.shape == (B_FULL, S_FULL, D) and weight.shape == (D, D)
    nc = _get_nc()
    xf = np.asarray(x, dtype=np.float32).reshape(M_FULL, D)
    wv = np.asarray(weight, dtype=np.float32)
    wdv = _prep_w(wv)
    h1 = np.asarray(_h128_np())
    hh = np.concatenate([h1, -h1], axis=0)
    in_maps = [
        {"xd": _prep_x(xf[c * M_CORE:(c + 1) * M_CORE]),
         "wd": wdv, "h": hh}
        for c in range(N_CORES)
    ]
    res = run_bass_kernel_spmd(nc, in_maps, core_ids=list(range(N_CORES)),
                               trace=trace)
    yv = np.concatenate([_unprep_y(r["yd"]) for r in res.results], axis=0)
    return yv.reshape(B_FULL, S_FULL, D), res


def kernel(x, weight):
    return run(x, weight)[0]
